# revision 1
# baseline (speedup 1.0000x reference)
"""Trainium2 Bass kernel for nn_MeshDeformationBlock (GNN message passing).

Data-parallel over batch: 2 batches per core, 8 cores.  Math rewrite:
  out = g@P0 + (A g)@P1 + (A^2 g)@P2 + (A^3 g)@P3      (biases are zero)
with g = bilinear(img, pos) + vertex_padded, A the symmetric edge operator,
P0..P3 host-precomputed 128x128 weight products.  Device work: Ant dma_gather
(bf16, 512B two-batch-interleaved rows, 4 SWDGE queues) + DVE plane adds via
degree-(d,a) subclass chunking; final combine = PE matmuls on xbar-transposed
reads, fp32 PSUM.
"""

import sys
import numpy as np
import ml_dtypes

sys.path.insert(0, "/opt/trn_rl_repo")

bf16 = ml_dtypes.bfloat16

B, V, C, H, W = 16, 40000, 128, 56, 56
NCORES = 8
NB = 2
TOKCAP = 4096
CVB = 2048
MIN_GROUP = 256
NPIX = 3329


# ---------------------------------------------------------------- host plan

def _build_graph_plan(edges):
    e = np.asarray(edges).astype(np.int64)
    src = np.concatenate([e[:, 1], e[:, 0]])
    dst = np.concatenate([e[:, 0], e[:, 1]])
    deg = np.bincount(dst, minlength=V).astype(np.int64)

    order = np.argsort(dst, kind="stable")
    nbr_flat = src[order]
    rowptr = np.zeros(V + 1, np.int64)
    rowptr[1:] = np.cumsum(deg)

    dmax = int(deg.max())
    counts_by_d = np.bincount(deg, minlength=dmax + 1)
    cum = np.cumsum(counts_by_d)
    dstar = int(np.searchsorted(cum, 18000))
    halfbit = deg <= dstar

    nbr_low = halfbit[nbr_flat]
    a_of = np.zeros(V, np.int64)
    np.add.at(a_of, dst[order], nbr_low.astype(np.int64))

    keys = {}
    dd, aa = deg, a_of
    for v in range(V):
        keys.setdefault((int(dd[v]), int(aa[v])), []).append(v)
    low_keys = sorted(k for k in keys if k[0] <= dstar)
    high_keys = sorted(k for k in keys if k[0] > dstar)

    def merge(klist):
        groups, cur, ca, cb = [], [], 0, 0
        for (d, a) in klist:
            cur.extend(keys[(d, a)])
            ca = max(ca, a)
            cb = max(cb, d - a)
            if len(cur) >= MIN_GROUP:
                groups.append((cur, ca, cb))
                cur, ca, cb = [], 0, 0
        if cur:
            groups.append((cur, ca, cb))
        return groups

    low_groups = [([], 0, 0)] + merge(low_keys)
    high_groups = merge(high_keys) + [([], 0, 0)]

    rowpos = np.full(V, -1, np.int64)
    group_meta = []
    pos = 0
    half_rows = None
    for side, groups in ((0, low_groups), (1, high_groups)):
        if side == 1:
            half_rows = pos
        for vs, A, Bn in groups:
            nreal = len(vs)
            nslots = max(128, -(-max(nreal, 1) // 128) * 128)
            if nreal:
                rowpos[np.array(vs, np.int64)] = pos + np.arange(nreal)
            group_meta.append((pos, nslots, A, Bn))
            pos += nslots
    Vp = -(-pos // 512) * 512
    if Vp > pos:
        group_meta.append((pos, Vp - pos, 0, 0))
    assert half_rows is not None
    assert half_rows < 32768 and (Vp - half_rows) < 32768, (half_rows, Vp)

    ZL, ZH = 0, Vp - 1
    vert_at = np.full(Vp, -1, np.int64)
    vert_at[rowpos[rowpos >= 0] if False else rowpos] = np.arange(V)

    tok_stream = []
    chunks = []
    off = 0
    for base, nslots, A, Bn in group_meta:
        D = A + Bn
        if D == 0:
            chunks.append((base, nslots, 0, 0, 0, 0))
            continue
        gv = max(128, (TOKCAP // D) // 128 * 128)
        for cb in range(base, base + nslots, gv):
            cg = min(gv, base + nslots - cb)
            lo = np.full((A, cg), ZL, np.int64)
            hi = np.full((Bn, cg), ZH - half_rows, np.int64)
            for u in range(cg):
                v = vert_at[cb + u]
                if v < 0:
                    continue
                ns = nbr_flat[rowptr[v]:rowptr[v + 1]]
                rp = rowpos[ns]
                rlo = rp[rp < half_rows]
                rhi = rp[rp >= half_rows] - half_rows
                lo[: len(rlo), u] = rlo
                hi[: len(rhi), u] = rhi
            off_lo = off
            tok_stream.append(lo.reshape(-1))
            off += A * cg
            off_hi = off
            tok_stream.append(hi.reshape(-1))
            off += Bn * cg
            chunks.append((cb, cg, A, Bn, off_lo, off_hi))
    # storage map: within each chunk rows are p-major (SBUF partition-contiguous)
    log2store = np.empty(Vp, np.int64)
    for (cb, cg, A, Bn, _ol, _oh) in chunks:
        nvb = cg // 128
        q = np.arange(cg)
        log2store[cb + q] = cb + (q % 128) * nvb + q // 128
    tok_l = (np.concatenate(tok_stream) if tok_stream else np.zeros(0, np.int64))
    # token values are logical rows (low: absolute; high: logical-HALF).
    # remap to storage rows.
    # rebuild with absolute logical values to remap, then re-split by half.
    tok_abs = []
    off2 = 0
    for (cb, cg, A, Bn, off_lo, off_hi) in chunks:
        if A + Bn == 0:
            continue
        tok_abs.append(log2store[tok_l[off_lo:off_lo + A * cg]])
        tok_abs.append(log2store[tok_l[off_hi:off_hi + Bn * cg] + half_rows] - half_rows)
    tok = (np.concatenate(tok_abs) if tok_abs else np.zeros(0, np.int64)).astype(np.int16)
    if len(tok) % 128:
        tok = np.concatenate([tok, np.zeros(128 - len(tok) % 128, np.int16)])

    return dict(rowpos=rowpos, vert_at=vert_at, Vp=Vp, half=half_rows,
                chunks=chunks, tok=tok, log2store=log2store)


def _wrap16(stream):
    n = len(stream)
    assert n % 16 == 0
    w = stream.reshape(n // 16, 16).T
    return np.ascontiguousarray(np.tile(w, (8, 1))).astype(np.int16)


def _bilinear_host(plan, pos_b):
    Vp = plan["Vp"]
    vert_at = plan["vert_at"]
    x = (pos_b[:, 0] + 1.0) * 0.5 * (W - 1)
    y = (pos_b[:, 1] + 1.0) * 0.5 * (H - 1)
    x0 = np.floor(x)
    y0 = np.floor(y)
    wx1 = (x - x0).astype(np.float32)
    wx0 = 1.0 - wx1
    wy1 = (y - y0).astype(np.float32)
    wy0 = 1.0 - wy1
    x0 = np.clip(x0.astype(np.int64), 0, W - 1)
    y0 = np.clip(y0.astype(np.int64), 0, H - 1)

    idxA = np.zeros(Vp, np.int64)
    idxB = np.zeros(Vp, np.int64)
    wA = np.zeros((Vp, 2), np.float32)
    wB = np.zeros((Vp, 2), np.float32)
    mask = vert_at >= 0
    vs = vert_at[mask]
    idxA[mask] = y0[vs] * W + x0[vs]
    idxB[mask] = np.minimum(y0[vs] + 1, H - 1) * W + x0[vs]
    wA[mask, 0] = wx0[vs] * wy0[vs]
    wA[mask, 1] = wx1[vs] * wy0[vs]
    wB[mask, 0] = wx0[vs] * wy1[vs]
    wB[mask, 1] = wx1[vs] * wy1[vs]

    # permute logical->storage, then emit gather streams/weights so that
    # phase-B slot (p,u) of block r0 holds the vertex at storage row r0+p*16+u:
    # the bilinear gather writes slot (p,u) from token position u*128+p, so the
    # token stream must be in colmajor order of the storage-blocked layout.
    l2s = plan["log2store"]
    idxA_s = np.zeros(Vp, np.int64); idxA_s[l2s] = idxA
    idxB_s = np.zeros(Vp, np.int64); idxB_s[l2s] = idxB
    wA_s = np.zeros((Vp, 2), np.float32); wA_s[l2s] = wA
    wB_s = np.zeros((Vp, 2), np.float32); wB_s[l2s] = wB
    stream = []
    for r0 in range(0, Vp, CVB):
        cv = min(CVB, Vp - r0)
        nv = cv // 128
        t = np.arange(cv)
        rows = r0 + (t % 128) * nv + t // 128
        stream.append(idxA_s[rows])
        stream.append(idxB_s[rows])
    stream = np.concatenate(stream).astype(np.int16)
    wAe = np.zeros((Vp, 2, 128), bf16)
    wBe = np.zeros((Vp, 2, 128), bf16)
    wAe[:] = wA_s.astype(bf16)[:, :, None]
    wBe[:] = wB_s.astype(bf16)[:, :, None]
    return _wrap16(stream), wAe.reshape(Vp, 256), wBe.reshape(Vp, 256)


# ---------------------------------------------------------------- device

def _build_kernel(plan):
    import concourse.bacc as bacc
    import concourse.mybir as mybir
    from concourse.tile import TileContext

    Vp, half = plan["Vp"], plan["half"]
    chunks = plan["chunks"]
    TOK = len(plan["tok"])

    nc = bacc.Bacc("TRN2", target_bir_lowering=False, debug=False,
                   num_swdge_queues=4)
    dt = mybir.dt

    imgp = nc.dram_tensor("imgp", [NB, NPIX, 256], dt.bfloat16, kind="ExternalInput")
    vpadp = nc.dram_tensor("vpadp", [Vp, 256], dt.bfloat16, kind="ExternalInput")
    bidx = nc.dram_tensor("bidx", [NB, 128, (2 * Vp) // 16], dt.int16, kind="ExternalInput")
    wAe = nc.dram_tensor("wAe", [NB, Vp, 256], dt.bfloat16, kind="ExternalInput")
    wBe = nc.dram_tensor("wBe", [NB, Vp, 256], dt.bfloat16, kind="ExternalInput")
    aidx = nc.dram_tensor("aidx", [128, TOK // 16], dt.int16, kind="ExternalInput")
    Pmat = nc.dram_tensor("Pmat", [4, 128, 128], dt.bfloat16, kind="ExternalInput")
    outcm = nc.dram_tensor("outcm", [NB, 128, Vp], dt.float32, kind="ExternalOutput")

    g_t = nc.dram_tensor("g_t", [Vp, 256], dt.bfloat16)
    a_t = [nc.dram_tensor(f"a{r}_t", [Vp, 256], dt.bfloat16) for r in range(3)]

    def cm(dram_rows):
        return dram_rows.rearrange("(p u) e -> p u e", p=128)

    qn = [0]
    with TileContext(nc) as tc:
        with tc.tile_pool(name="res", bufs=1) as res:
            aidx_sb = res.tile([128, TOK // 16], dt.int16)
            nc.sync.dma_start(out=aidx_sb[:], in_=aidx[:, :])
            P_sb = res.tile([128, 4, 128], dt.bfloat16)
            nc.sync.dma_start(out=P_sb[:], in_=Pmat[:, :, :].rearrange("k p m -> p k m"))
            zt = res.tile([128, 256], dt.bfloat16)
            nc.vector.memset(zt[:], 0.0)

            # ------------- phase B: g = bilinear + vpad -------------
            with tc.tile_pool(name="bil", bufs=2) as bilp:
                bidx_sb = []
                for b in range(NB):
                    t = res.tile([128, (2 * Vp) // 16], dt.int16, tag=f"bi{b}")
                    nc.sync.dma_start(out=t[:], in_=bidx[b, :, :])
                    bidx_sb.append(t)
                for r0 in range(0, Vp, CVB):
                    cv = min(CVB, Vp - r0)
                    nv = cv // 128
                    gst = bilp.tile([128, nv, 2, 128], dt.bfloat16, tag="gst")
                    for b in range(NB):
                        taps = bilp.tile([128, 2 * nv, 256], dt.bfloat16, tag="taps")
                        toff = 2 * r0
                        nc.gpsimd.dma_gather(
                            taps[:], imgp[b, :, :],
                            bidx_sb[b][:, toff // 16:(toff + 2 * cv) // 16],
                            2 * cv, 2 * cv, 256, single_packet=False,
                            queue_num=qn[0] % 4)
                        qn[0] += 1
                        wa = bilp.tile([128, nv, 256], dt.bfloat16, tag="wa")
                        wb = bilp.tile([128, nv, 256], dt.bfloat16, tag="wb")
                        nc.sync.dma_start(out=wa[:], in_=cm(wAe[b, r0:r0 + cv, :]))
                        nc.sync.dma_start(out=wb[:], in_=cm(wBe[b, r0:r0 + cv, :]))
                        vp = bilp.tile([128, nv, 2, 128], dt.bfloat16, tag="vp")
                        nc.sync.dma_start(
                            out=vp[:],
                            in_=cm(vpadp[r0:r0 + cv, :])
                            .rearrange("p u (x c) -> p u x c", x=2))
                        nc.vector.tensor_mul(out=taps[:, :nv, :],
                                             in0=taps[:, :nv, :], in1=wa[:])
                        nc.vector.tensor_mul(out=taps[:, nv:, :],
                                             in0=taps[:, nv:, :], in1=wb[:])
                        nc.vector.tensor_add(out=taps[:, :nv, :],
                                             in0=taps[:, :nv, :],
                                             in1=taps[:, nv:, :])
                        t4 = taps[:, :nv, :].rearrange("p a (x c) -> p a x c", x=2)
                        nc.vector.tensor_add(out=t4[:, :, 0, :],
                                             in0=t4[:, :, 0, :], in1=t4[:, :, 1, :])
                        nc.vector.tensor_add(out=gst[:, :, b, :],
                                             in0=t4[:, :, 0, :], in1=vp[:, :, b, :])
                    nc.sync.dma_start(
                        out=cm(g_t.ap()[r0:r0 + cv, :]),
                        in_=gst[:].rearrange("p u x c -> p u (x c)"))

            # ------------- phase C: a_{r+1} = A a_r -----------------
            with (tc.tile_pool(name="gb", bufs=6) as gbp,
                  tc.tile_pool(name="ac", bufs=4) as acp):
                for r in range(3):
                    src = g_t if r == 0 else a_t[r - 1]
                    dst = a_t[r]
                    for (base, gv, A, Bn, off_lo, off_hi) in chunks:
                        D = A + Bn
                        if D == 0:
                            for u0 in range(0, gv, 128):
                                nc.sync.dma_start(
                                    out=cm(dst.ap()[base + u0:base + u0 + 128, :]),
                                    in_=zt[:].rearrange("p (u e) -> p u e", u=1))
                            continue
                        buf = gbp.tile([128, (gv * D) // 128, 256], dt.bfloat16,
                                       tag="gb")
                        if A:
                            nc.gpsimd.dma_gather(
                                buf[:, :(gv * A) // 128, :], src.ap()[:, :],
                                aidx_sb[:, off_lo // 16:(off_lo + gv * A) // 16],
                                gv * A, gv * A, 256, single_packet=False,
                                queue_num=qn[0] % 4)
                            qn[0] += 1
                        if Bn:
                            nc.gpsimd.dma_gather(
                                buf[:, (gv * A) // 128:, :], src.ap()[half:, :],
                                aidx_sb[:, off_hi // 16:(off_hi + gv * Bn) // 16],
                                gv * Bn, gv * Bn, 256, single_packet=False,
                                queue_num=qn[0] % 4)
                            qn[0] += 1
                        nvb = gv // 128
                        if D == 1:
                            nc.sync.dma_start(out=cm(dst.ap()[base:base + gv, :]),
                                              in_=buf[:, :nvb, :])
                            continue
                        acc = acp.tile([128, nvb, 256], dt.bfloat16, tag="acc")
                        nc.vector.tensor_add(out=acc[:], in0=buf[:, :nvb, :],
                                             in1=buf[:, nvb:2 * nvb, :])
                        for k in range(2, D):
                            nc.vector.tensor_add(
                                out=acc[:], in0=acc[:],
                                in1=buf[:, k * nvb:(k + 1) * nvb, :])
                        nc.sync.dma_start(out=cm(dst.ap()[base:base + gv, :]),
                                          in_=acc[:])

            # ------------- phase D: combine -------------------------
            with (tc.tile_pool(name="dp", bufs=3) as dp,
                  tc.tile_pool(name="ps", bufs=4, space="PSUM") as psp):
                for b in range(NB):
                    for blk in range(0, Vp, 512):
                        ps = psp.tile([128, 512], dt.float32)
                        for k, T in enumerate([g_t, a_t[0], a_t[1], a_t[2]]):
                            xT = dp.tile([128, 512], dt.bfloat16, tag=f"x{k}")
                            nc.sync.dma_start(
                                out=xT[:],
                                in_=T.ap()[blk:blk + 512, :]
                                .rearrange("r (x c) -> r x c", x=2)[:, b, :],
                                transpose=True)
                            nc.tensor.matmul(out=ps[:],
                                             lhsT=P_sb[:, k, :],
                                             rhs=xT[:], start=(k == 0),
                                             stop=(k == 3))
                        ot = dp.tile([128, 512], dt.float32, tag="ot")
                        nc.scalar.activation(
                            out=ot[:], in_=ps[:],
                            func=mybir.ActivationFunctionType.Copy)
                        nc.sync.dma_start(out=outcm[b, :, blk:blk + 512],
                                          in_=ot[:])

    nc.compile()
    return nc


# ---------------------------------------------------------------- entry

def _make_in_maps(plan, inputs):
    Vp = plan["Vp"]
    M = [np.asarray(inputs[f"w0_{i}"], np.float64) for i in (1, 2, 3)]
    N = [np.asarray(inputs[f"w1_{i}"], np.float64) for i in (1, 2, 3)]
    P0 = M[0] + M[0] @ M[1] @ M[2]
    P1 = N[0] + N[0] @ M[1] @ M[2] + M[0] @ N[1] @ M[2] + M[0] @ M[1] @ N[2]
    P2 = N[0] @ N[1] @ M[2] + N[0] @ M[1] @ N[2] + M[0] @ N[1] @ N[2]
    P3 = N[0] @ N[1] @ N[2]
    Pm = np.ascontiguousarray(np.stack([P0, P1, P2, P3]).astype(bf16))

    img = np.asarray(inputs["img_features"], np.float32)
    pos = np.asarray(inputs["vertex_position"], np.float32)
    vpad = np.asarray(inputs["vertex_padded"], np.float32)

    imgr = img.transpose(0, 2, 3, 1).reshape(B, H * W, C).astype(bf16)
    imgpad = np.zeros((B, NPIX + 1, C), bf16)
    imgpad[:, :H * W] = imgr
    imgp_all = np.concatenate([imgpad[:, :NPIX], imgpad[:, 1:NPIX + 1]], axis=2)

    aidx_w = _wrap16(plan["tok"])
    mask = plan["vert_at"] >= 0
    vs = plan["vert_at"][mask]

    in_maps = []
    for core in range(NCORES):
        bs = [NB * core + i for i in range(NB)]
        bidx_l, wAe_l, wBe_l = [], [], []
        for b in bs:
            bi, wa, wb = _bilinear_host(plan, pos[b])
            bidx_l.append(bi)
            wAe_l.append(wa)
            wBe_l.append(wb)
        vpadp = np.zeros((Vp, 2, 128), bf16)
        srows = plan["log2store"][plan["rowpos"]]
        for i, b in enumerate(bs):
            vpadp[srows, i, :] = vpad[b].astype(bf16)
        in_maps.append({
            "imgp": np.ascontiguousarray(np.stack([imgp_all[b] for b in bs])),
            "vpadp": np.ascontiguousarray(vpadp.reshape(Vp, 256)),
            "bidx": np.ascontiguousarray(np.stack(bidx_l)),
            "wAe": np.ascontiguousarray(np.stack(wAe_l)),
            "wBe": np.ascontiguousarray(np.stack(wBe_l)),
            "aidx": aidx_w,
            "Pmat": Pm,
        })
    return in_maps


_CACHE = {}


def kernel(**inputs):
    from concourse import bass_utils

    plan = _build_graph_plan(inputs["edges"])
    in_maps = _make_in_maps(plan, inputs)
    key = "nc"
    if key not in _CACHE:
        _CACHE[key] = _build_kernel(plan)
    nc = _CACHE[key]
    res = bass_utils.run_bass_kernel_spmd(nc, in_maps, core_ids=list(range(NCORES)))

    srows = plan["log2store"][plan["rowpos"]]
    out = np.zeros((B, V, C), np.float32)
    for core in range(NCORES):
        oc = res.results[core]["outcm"]
        for i in range(NB):
            out[NB * core + i] = oc[i][:, srows].T
    return out



# revision 7
# speedup vs baseline: 1.0348x; 1.0348x over previous
"""Trainium2 Bass kernel for nn_MeshDeformationBlock (GNN message passing).

Data-parallel over batch: 2 batches per core, 8 cores.  Math rewrite:
  out = g@P0 + (A g)@P1 + (A^2 g)@P2 + (A^3 g)@P3      (biases are zero)
with g = bilinear(img, pos) + vertex_padded, A the symmetric edge operator,
P0..P3 host-precomputed 128x128 weight products.

Layout: vertices sorted by (low-nbr-count, high-nbr-count) into uniform
256-slot chunks; gathers batched into ~4096-token granules round-robined
over 4 SWDGE queues (one queue per in-flight granule — concurrent
transpose-gathers race on HW, so none are used).  Bilinear uses a
host-built 4-pixel table (one 1KB token per vertex) with compact
per-vertex weights broadcast on-chip via stride-0 APs.  Every state
table is mirrored channel-major ([2,128,Vp]) at production time via DVE
32x32 stream-transposes + block-permuting stores on the idle
Scalar/Sync DMA queues, so the final combine is plain contiguous loads
feeding PE matmuls with fp32 PSUM accumulation.
"""

import sys
import numpy as np
import ml_dtypes

sys.path.insert(0, "/opt/trn_rl_repo")

bf16 = ml_dtypes.bfloat16

B, V, C, H, W = 16, 40000, 128, 56, 56
NCORES = 8
NB = 2
CS = 256          # chunk slots
NVB = CS // 128
GR_CAP = 4096     # max tokens per gather granule
CVB = 2048        # bilinear block rows
NPIX = H * W


# ---------------------------------------------------------------- host plan

def _build_graph_plan(edges):
    e = np.asarray(edges).astype(np.int64)
    src = np.concatenate([e[:, 1], e[:, 0]])
    dst = np.concatenate([e[:, 0], e[:, 1]])
    deg = np.bincount(dst, minlength=V).astype(np.int64)

    order = np.argsort(dst, kind="stable")
    nbr_flat = src[order]
    rowptr = np.zeros(V + 1, np.int64)
    rowptr[1:] = np.cumsum(deg)

    counts_by_d = np.bincount(deg)
    cum = np.cumsum(counts_by_d)
    dstar = int(np.searchsorted(cum, 18000))
    halfbit = deg <= dstar

    a_of = np.zeros(V, np.int64)
    np.add.at(a_of, dst[order], halfbit[nbr_flat].astype(np.int64))
    b_of = deg - a_of

    chunks = []       # (base, A, B, n_real)
    rowpos = np.full(V, -1, np.int64)
    chunk_slot_vs = []
    pos = 0
    half = None
    for side in (0, 1):
        # leading all-zero chunk per half: dummy/padding tokens point at its
        # first row, so their contributions vanish.
        chunks.append((pos, 0, 0, 0))
        chunk_slot_vs.append(np.zeros(0, np.int64))
        pos += CS
        vs = np.nonzero(halfbit if side == 0 else ~halfbit)[0]
        o = np.lexsort((b_of[vs], a_of[vs]))
        vs = vs[o]
        n = len(vs)
        for i in range(0, n, CS):
            cvs = vs[i:i + CS]
            q = np.arange(len(cvs))
            rowpos[cvs] = pos + (q % 128) * NVB + q // 128
            chunks.append((pos, int(a_of[cvs].max()), int(b_of[cvs].max()),
                           len(cvs)))
            chunk_slot_vs.append(cvs)
            pos += CS
        if side == 0:
            half = pos
    assert half is not None and half < 32768 and (pos - half) < 32768
    Vp = -(-pos // 512) * 512
    if Vp > pos:
        chunks.append((pos, 0, 0, 0))
        chunk_slot_vs.append(np.zeros(0, np.int64))
        pos = Vp

    low_nbrs, high_nbrs = {}, {}
    for v in range(V):
        ns = nbr_flat[rowptr[v]:rowptr[v + 1]]
        lb = halfbit[ns]
        low_nbrs[v] = rowpos[ns[lb]]
        high_nbrs[v] = rowpos[ns[~lb]] - half

    # token streams + per-stream granule packing.  A granule is one gather
    # call (<= GR_CAP tokens); chunk blocks never straddle granules.
    granules = []              # (aidx_off, ntok)
    tok_parts = []
    gmap = {}                  # (stream, ci) -> (granule_id, off_in_granule)
    cur_items = {0: [], 1: []}
    cur_toks = {0: [], 1: []}
    cur_sz = {0: 0, 1: 0}

    def close(stream):
        if not cur_sz[stream]:
            return
        off = sum(len(t) for t in tok_parts)
        gi = len(granules)
        granules.append((off, cur_sz[stream]))
        tok_parts.extend(cur_toks[stream])
        for ci, off_in in cur_items[stream]:
            gmap[(stream, ci)] = (gi, off_in)
        cur_items[stream], cur_toks[stream] = [], []
        cur_sz[stream] = 0

    for ci, (base, A, Bn, nreal) in enumerate(chunks):
        cvs = chunk_slot_vs[ci]
        for stream, D, nbrs in ((0, A, low_nbrs), (1, Bn, high_nbrs)):
            if D == 0:
                continue
            blk = np.zeros((D, CS), np.int64)
            for q, v in enumerate(cvs):
                r = nbrs[v]
                blk[: len(r), q] = r
            if cur_sz[stream] and cur_sz[stream] + D * CS > GR_CAP:
                close(stream)
            cur_items[stream].append((ci, cur_sz[stream]))
            cur_toks[stream].append(blk.reshape(-1))
            cur_sz[stream] += D * CS
    close(0)
    close(1)

    tok = (np.concatenate(tok_parts) if tok_parts else np.zeros(0, np.int64))
    assert len(tok) % 128 == 0
    tok = tok.astype(np.int16)

    chunk_meta = []
    for ci, (base, A, Bn, nreal) in enumerate(chunks):
        lo = gmap.get((0, ci))
        hi = gmap.get((1, ci))
        chunk_meta.append((base, A, Bn,
                           lo[0] if lo else -1, lo[1] if lo else 0,
                           hi[0] if hi else -1, hi[1] if hi else 0))

    return dict(rowpos=rowpos, Vp=Vp, half=half, chunks=chunk_meta,
                granules=granules, tok=tok)


def _wrap16(stream):
    n = len(stream)
    assert n % 16 == 0
    w = stream.reshape(n // 16, 16).T
    return np.ascontiguousarray(np.tile(w, (8, 1))).astype(np.int16)


def _bilinear_host(plan, pos_b):
    """Per-batch: pixel-table token stream (block-colmajor order) and compact
    4-tap weights [Vp, 4] in storage-row order."""
    Vp = plan["Vp"]
    rowpos = plan["rowpos"]
    x = (pos_b[:, 0] + 1.0) * 0.5 * (W - 1)
    y = (pos_b[:, 1] + 1.0) * 0.5 * (H - 1)
    x0 = np.floor(x)
    y0 = np.floor(y)
    wx1 = (x - x0).astype(np.float32)
    wx0 = 1.0 - wx1
    wy1 = (y - y0).astype(np.float32)
    wy0 = 1.0 - wy1
    x0 = np.clip(x0.astype(np.int64), 0, W - 1)
    y0 = np.clip(y0.astype(np.int64), 0, H - 1)

    pixidx = np.zeros(Vp, np.int64)
    w4 = np.zeros((Vp, 4), np.float32)
    pixidx[rowpos] = y0 * W + x0
    w4[rowpos, 0] = wx0 * wy0
    w4[rowpos, 1] = wx1 * wy0
    w4[rowpos, 2] = wx0 * wy1
    w4[rowpos, 3] = wx1 * wy1

    stream = []
    for r0 in range(0, Vp, CVB):
        cv = min(CVB, Vp - r0)
        nv = cv // 128
        t = np.arange(cv)
        rows = r0 + (t % 128) * nv + t // 128
        stream.append(pixidx[rows])
    stream = np.concatenate(stream).astype(np.int16)
    return _wrap16(stream), w4.astype(bf16)


# ---------------------------------------------------------------- device

def _build_kernel(plan):
    import concourse.bacc as bacc
    import concourse.mybir as mybir
    from concourse.tile import TileContext

    Vp, half = plan["Vp"], plan["half"]
    chunks = plan["chunks"]
    granules = plan["granules"]
    TOK = len(plan["tok"])

    nc = bacc.Bacc("TRN2", target_bir_lowering=False, debug=False,
                   num_swdge_queues=4)
    dt = mybir.dt

    img4 = nc.dram_tensor("img4", [NB, NPIX, 512], dt.bfloat16,
                          kind="ExternalInput")
    vpadp = nc.dram_tensor("vpadp", [Vp, 256], dt.bfloat16,
                           kind="ExternalInput")
    bidx = nc.dram_tensor("bidx", [NB, 128, Vp // 16], dt.int16,
                          kind="ExternalInput")
    w4t = nc.dram_tensor("w4t", [NB, Vp, 4], dt.bfloat16,
                         kind="ExternalInput")
    aidx = nc.dram_tensor("aidx", [128, TOK // 16], dt.int16,
                          kind="ExternalInput")
    Pmat = nc.dram_tensor("Pmat", [4, 128, 128], dt.bfloat16,
                          kind="ExternalInput")
    outcm = nc.dram_tensor("outcm", [NB, 128, Vp], dt.float32,
                           kind="ExternalOutput")

    g_t = nc.dram_tensor("g_t", [Vp, 256], dt.bfloat16)
    a_t = [nc.dram_tensor(f"a{r}_t", [Vp, 256], dt.bfloat16)
           for r in range(2)]
    # channel-major mirrors: xc[k][x, c, row] = table_k[row, x*128+c]
    xc = [nc.dram_tensor(f"xc{k}", [2, 128, Vp], dt.bfloat16)
          for k in range(4)]

    def cm(dram_rows):
        return dram_rows.rearrange("(p u) e -> p u e", p=128)

    def cm_stores(eng, dram_cm, acc_t, base, u):
        """acc_t [128, 2, 4, 32, u] (x, jc, s, u) -> dram_cm[:, :, rows]"""
        su = 32 * u
        for i in range(4):
            out_ap = (dram_cm.ap()[:, :, base + su * i: base + su * (i + 1)]
                      .rearrange("x (jc r) (s u) -> r (x jc) (s u)",
                                 jc=4, s=32))
            in_ap = (acc_t[32 * i:32 * (i + 1)]
                     .rearrange("p x jc s u -> p (x jc) (s u)"))
            eng.dma_start(out=out_ap, in_=in_ap)

    qn = [0]
    with TileContext(nc) as tc:
        with tc.tile_pool(name="res", bufs=1) as res:
            aidx_sb = res.tile([128, TOK // 16], dt.int16)
            nc.sync.dma_start(out=aidx_sb[:], in_=aidx[:, :])
            P_sb = res.tile([128, 4, 128], dt.bfloat16)
            nc.sync.dma_start(out=P_sb[:],
                              in_=Pmat[:, :, :].rearrange("k p m -> p k m"))
            zt = res.tile([128, NVB, 256], dt.bfloat16)
            nc.vector.memset(zt[:], 0.0)
            zt_c = res.tile([128, 2, 4, 32, NVB], dt.bfloat16)
            nc.vector.memset(zt_c[:], 0.0)

            # ------------- phase B: g = bilinear + vpad -------------
            with (tc.tile_pool(name="bil", bufs=2) as bilp,
                  tc.tile_pool(name="bidxp", bufs=1) as bidxp):
                bidx_sb = []
                for b in range(NB):
                    t = bidxp.tile([128, Vp // 16], dt.int16, tag=f"bi{b}")
                    nc.sync.dma_start(out=t[:], in_=bidx[b, :, :])
                    bidx_sb.append(t)
                for r0 in range(0, Vp, CVB):
                    cv = min(CVB, Vp - r0)
                    nv = cv // 128
                    gst = bilp.tile([128, nv, 2, 128], dt.bfloat16, tag="gst")
                    vp = bilp.tile([128, nv, 2, 128], dt.bfloat16, tag="vp")
                    nc.sync.dma_start(
                        out=vp[:],
                        in_=cm(vpadp[r0:r0 + cv, :])
                        .rearrange("p u (x c) -> p u x c", x=2))
                    for b in range(NB):
                        taps = bilp.tile([128, nv, 4, 128], dt.bfloat16,
                                         tag=f"taps{b}")
                        nc.gpsimd.dma_gather(
                            taps[:].rearrange("p u x c -> p u (x c)"),
                            img4[b, :, :],
                            bidx_sb[b][:, r0 // 16:(r0 + cv) // 16],
                            cv, cv, 512, single_packet=False,
                            queue_num=qn[0] % 4)
                        qn[0] += 1
                        w4sb = bilp.tile([128, nv, 4], dt.bfloat16,
                                         tag=f"w4{b}")
                        nc.sync.dma_start(out=w4sb[:],
                                          in_=cm(w4t[b, r0:r0 + cv, :]))
                        w4b = (w4sb[:].rearrange("p u x -> p (u x)")
                               .unsqueeze(2).broadcast_to((128, nv * 4, 128)))
                        t3 = taps[:].rearrange("p u x c -> p (u x) c")
                        nc.vector.tensor_mul(out=t3, in0=t3, in1=w4b)
                        tf = taps[:].rearrange("p u x c -> p u (x c)")
                        nc.vector.tensor_add(out=tf[:, :, 0:256],
                                             in0=tf[:, :, 0:256],
                                             in1=tf[:, :, 256:512])
                        nc.vector.tensor_add(out=gst[:, :, b, :],
                                             in0=tf[:, :, 0:128],
                                             in1=tf[:, :, 128:256])
                        nc.vector.tensor_add(out=gst[:, :, b, :],
                                             in0=gst[:, :, b, :],
                                             in1=vp[:, :, b, :])
                    nc.sync.dma_start(
                        out=cm(g_t.ap()[r0:r0 + cv, :]),
                        in_=gst[:].rearrange("p u x c -> p u (x c)"))
                    gst_c = bilp.tile([128, 2, 4, 32, nv], dt.bfloat16,
                                      tag="gstc")
                    nc.vector.transpose(
                        out=gst_c[:].rearrange("p x jc s u -> p u x jc s"),
                        in_=gst[:].rearrange("p u x (jc s) -> p u x jc s",
                                             jc=4))
                    cm_stores(nc.scalar, xc[0], gst_c[:], r0, nv)

            # ------------- phase C: a_{r+1} = A a_r -----------------
            GRB = GR_CAP // 128
            with (tc.tile_pool(name="gb", bufs=6) as gbp,
                  tc.tile_pool(name="ac", bufs=8) as acp):
                for r in range(3):
                    src = g_t if r == 0 else a_t[r - 1]
                    dst = a_t[r] if r < 2 else None
                    dcm = xc[r + 1]
                    issued = {}
                    for (base, A, Bn, lo_g, lo_off, hi_g, hi_off) in chunks:
                        for gidx, is_hi in ((lo_g, 0), (hi_g, 1)):
                            if gidx < 0 or gidx in issued:
                                continue
                            goff, gn = granules[gidx]
                            buf = gbp.tile([128, GRB, 256], dt.bfloat16,
                                           tag="gb")
                            src_ap = (src.ap()[half:, :] if is_hi
                                      else src.ap()[:, :])
                            nc.gpsimd.dma_gather(
                                buf[:, :gn // 128, :], src_ap,
                                aidx_sb[:, goff // 16:(goff + gn) // 16],
                                gn, gn, 256, single_packet=False,
                                queue_num=qn[0] % 4)
                            qn[0] += 1
                            issued[gidx] = buf
                        D = A + Bn
                        if D == 0:
                            if dst is not None:
                                nc.sync.dma_start(
                                    out=cm(dst.ap()[base:base + CS, :]),
                                    in_=zt[:])
                            cm_stores(nc.scalar, dcm, zt_c[:], base, NVB)
                            continue
                        slices = []
                        if A:
                            bl = issued[lo_g]
                            o = lo_off // 128
                            slices += [bl[:, o + k * NVB:o + (k + 1) * NVB, :]
                                       for k in range(A)]
                        if Bn:
                            bh = issued[hi_g]
                            o = hi_off // 128
                            slices += [bh[:, o + k * NVB:o + (k + 1) * NVB, :]
                                       for k in range(Bn)]
                        if D == 1:
                            acc = slices[0]
                        else:
                            acc = acp.tile([128, NVB, 256], dt.bfloat16,
                                           tag="acc")
                            nc.vector.tensor_add(out=acc[:], in0=slices[0],
                                                 in1=slices[1])
                            for s in slices[2:]:
                                nc.vector.tensor_add(out=acc[:], in0=acc[:],
                                                     in1=s)
                            acc = acc[:]
                        if dst is not None:
                            nc.sync.dma_start(
                                out=cm(dst.ap()[base:base + CS, :]),
                                in_=acc)
                        acc_c = acp.tile([128, 2, 4, 32, NVB], dt.bfloat16,
                                         tag="accc")
                        nc.vector.transpose(
                            out=acc_c[:].rearrange("p x jc s u -> p u x jc s"),
                            in_=acc.rearrange("p u (x jc s) -> p u x jc s",
                                              x=2, jc=4))
                        cm_stores(nc.scalar, dcm, acc_c[:], base, NVB)

            # ------------- phase D: combine -------------------------
            with (tc.tile_pool(name="dp", bufs=3) as dp,
                  tc.tile_pool(name="ps", bufs=4, space="PSUM") as psp):
                for blk in range(0, Vp, 512):
                    for b in range(NB):
                        ps = psp.tile([128, 512], dt.float32)
                        for k in range(4):
                            xb = dp.tile([128, 512], dt.bfloat16,
                                         tag=f"x{k}b{b}")
                            eng = nc.sync if k < 2 else nc.scalar
                            eng.dma_start(
                                out=xb[:], in_=xc[k][b, :, blk:blk + 512])
                            nc.tensor.matmul(out=ps[:],
                                             lhsT=P_sb[:, k, :],
                                             rhs=xb[:],
                                             start=(k == 0), stop=(k == 3))
                        ot = dp.tile([128, 512], dt.float32, tag=f"ot{b}")
                        nc.scalar.activation(
                            out=ot[:], in_=ps[:],
                            func=mybir.ActivationFunctionType.Copy)
                        nc.sync.dma_start(out=outcm[b, :, blk:blk + 512],
                                          in_=ot[:])

    nc.compile()
    return nc


# ---------------------------------------------------------------- entry

def _make_in_maps(plan, inputs):
    Vp = plan["Vp"]
    rowpos = plan["rowpos"]
    M = [np.asarray(inputs[f"w0_{i}"], np.float64) for i in (1, 2, 3)]
    N = [np.asarray(inputs[f"w1_{i}"], np.float64) for i in (1, 2, 3)]
    P0 = M[0] + M[0] @ M[1] @ M[2]
    P1 = N[0] + N[0] @ M[1] @ M[2] + M[0] @ N[1] @ M[2] + M[0] @ M[1] @ N[2]
    P2 = N[0] @ N[1] @ M[2] + N[0] @ M[1] @ N[2] + M[0] @ N[1] @ N[2]
    P3 = N[0] @ N[1] @ N[2]
    Pm = np.ascontiguousarray(np.stack([P0, P1, P2, P3]).astype(bf16))

    img = np.asarray(inputs["img_features"], np.float32)
    pos = np.asarray(inputs["vertex_position"], np.float32)
    vpad = np.asarray(inputs["vertex_padded"], np.float32)

    # 4-pixel table per batch: row(y*W+x) = [f(y,x), f(y,x+1), f(y+1,x),
    # f(y+1,x+1)] with clamped borders (their taps always carry weight 0).
    F = img.transpose(0, 2, 3, 1)  # [B, H, W, C]
    ys, xs = np.mgrid[0:H, 0:W]
    yp = np.minimum(ys + 1, H - 1)
    xp = np.minimum(xs + 1, W - 1)
    img4_all = np.concatenate(
        [F[:, ys, xs], F[:, ys, xp], F[:, yp, xs], F[:, yp, xp]],
        axis=-1).reshape(B, NPIX, 512).astype(bf16)

    aidx_w = _wrap16(plan["tok"])

    in_maps = []
    for core in range(NCORES):
        bs = [NB * core + i for i in range(NB)]
        bidx_l, w4_l = [], []
        for b in bs:
            bi, w4 = _bilinear_host(plan, pos[b])
            bidx_l.append(bi)
            w4_l.append(w4)
        vpadp = np.zeros((Vp, 2, 128), bf16)
        for i, b in enumerate(bs):
            vpadp[rowpos, i, :] = vpad[b].astype(bf16)
        in_maps.append({
            "img4": np.ascontiguousarray(img4_all[bs]),
            "vpadp": np.ascontiguousarray(vpadp.reshape(Vp, 256)),
            "bidx": np.ascontiguousarray(np.stack(bidx_l)),
            "w4t": np.ascontiguousarray(np.stack(w4_l)),
            "aidx": aidx_w,
            "Pmat": Pm,
        })
    return in_maps


_CACHE = {}


def kernel(**inputs):
    from concourse import bass_utils

    plan = _build_graph_plan(inputs["edges"])
    in_maps = _make_in_maps(plan, inputs)
    key = "nc"
    if key not in _CACHE:
        _CACHE[key] = _build_kernel(plan)
    nc = _CACHE[key]
    res = bass_utils.run_bass_kernel_spmd(nc, in_maps,
                                          core_ids=list(range(NCORES)))

    out = np.zeros((B, V, C), np.float32)
    for core in range(NCORES):
        oc = res.results[core]["outcm"]
        for i in range(NB):
            out[NB * core + i] = oc[i][:, plan["rowpos"]].T
    return out


# revision 16
# speedup vs baseline: 1.1658x; 1.1265x over previous
"""Trainium2 Bass kernel for nn_MeshDeformationBlock (GNN message passing).

Data-parallel over batch: 2 batches per core, 8 cores.  Math rewrite:
  out = g@P0 + (A g)@P1 + (A^2 g)@P2 + (A^3 g)@P3      (biases are zero)
with g = bilinear(img, pos) + vertex_padded, A the symmetric edge operator,
P0..P3 host-precomputed 128x128 weight products.

Layout: vertices sorted by (low-nbr-count, high-nbr-count) into uniform
256-slot chunks; gathers batched into ~4096-token granules round-robined
over 4 SWDGE queues (one queue per in-flight granule — concurrent
transpose-gathers race on HW, so none are used).  Bilinear uses a
host-built 4-pixel table (one 1KB token per vertex) with compact
per-vertex weights broadcast on-chip via stride-0 APs.  Every state
table is mirrored channel-major ([2,128,Vp]) at production time via DVE
32x32 stream-transposes + block-permuting stores on the idle
Scalar/Sync DMA queues, so the final combine is plain contiguous loads
feeding PE matmuls with fp32 PSUM accumulation.
"""

import sys
import numpy as np
import ml_dtypes

sys.path.insert(0, "/opt/trn_rl_repo")

bf16 = ml_dtypes.bfloat16

B, V, C, H, W = 16, 40000, 128, 56, 56
NCORES = 8
NB = 2
CS = 256          # chunk slots
NVB = CS // 128
GR_CAP = 4096     # max tokens per gather granule
CVB = 2048        # bilinear block rows
NPIX = H * W


# ---------------------------------------------------------------- host plan

def _build_graph_plan(edges):
    e = np.asarray(edges).astype(np.int64)
    src = np.concatenate([e[:, 1], e[:, 0]])
    dst = np.concatenate([e[:, 0], e[:, 1]])
    deg = np.bincount(dst, minlength=V).astype(np.int64)

    order = np.argsort(dst, kind="stable")
    nbr_flat = src[order]
    rowptr = np.zeros(V + 1, np.int64)
    rowptr[1:] = np.cumsum(deg)

    counts_by_d = np.bincount(deg)
    cum = np.cumsum(counts_by_d)
    dstar = int(np.searchsorted(cum, 18000))
    halfbit = deg <= dstar

    a_of = np.zeros(V, np.int64)
    np.add.at(a_of, dst[order], halfbit[nbr_flat].astype(np.int64))
    b_of = deg - a_of

    chunks = []       # (base, A, B, n_real)
    rowpos = np.full(V, -1, np.int64)
    chunk_slot_vs = []
    pos = 0
    half = None
    for side in (0, 1):
        # leading all-zero chunk per half: dummy/padding tokens point at its
        # first row, so their contributions vanish.
        chunks.append((pos, 0, 0, 0))
        chunk_slot_vs.append(np.zeros(0, np.int64))
        pos += CS
        vs = np.nonzero(halfbit if side == 0 else ~halfbit)[0]
        o = np.lexsort((b_of[vs], a_of[vs]))
        vs = vs[o]
        n = len(vs)
        for i in range(0, n, CS):
            cvs = vs[i:i + CS]
            q = np.arange(len(cvs))
            rowpos[cvs] = pos + (q % 128) * NVB + q // 128
            chunks.append((pos, int(a_of[cvs].max()), int(b_of[cvs].max()),
                           len(cvs)))
            chunk_slot_vs.append(cvs)
            pos += CS
        if side == 0:
            half = pos
    assert half is not None and half < 32768 and (pos - half) < 32768
    Vp = -(-pos // 512) * 512
    if Vp > pos:
        chunks.append((pos, 0, 0, 0))
        chunk_slot_vs.append(np.zeros(0, np.int64))
        pos = Vp

    low_nbrs, high_nbrs = {}, {}
    for v in range(V):
        ns = nbr_flat[rowptr[v]:rowptr[v + 1]]
        lb = halfbit[ns]
        low_nbrs[v] = rowpos[ns[lb]]
        high_nbrs[v] = rowpos[ns[~lb]] - half

    # token streams + per-stream granule packing.  A granule is one gather
    # call (<= GR_CAP tokens); chunk blocks never straddle granules.
    granules = []              # (aidx_off, ntok)
    tok_parts = []
    gmap = {}                  # (stream, ci) -> (granule_id, off_in_granule)
    cur_items = {0: [], 1: []}
    cur_toks = {0: [], 1: []}
    cur_sz = {0: 0, 1: 0}

    def close(stream):
        if not cur_sz[stream]:
            return
        off = sum(len(t) for t in tok_parts)
        gi = len(granules)
        granules.append((off, cur_sz[stream]))
        tok_parts.extend(cur_toks[stream])
        for ci, off_in in cur_items[stream]:
            gmap[(stream, ci)] = (gi, off_in)
        cur_items[stream], cur_toks[stream] = [], []
        cur_sz[stream] = 0

    for ci, (base, A, Bn, nreal) in enumerate(chunks):
        cvs = chunk_slot_vs[ci]
        for stream, D, nbrs in ((0, A, low_nbrs), (1, Bn, high_nbrs)):
            if D == 0:
                continue
            blk = np.zeros((D, CS), np.int64)
            for q, v in enumerate(cvs):
                r = nbrs[v]
                blk[: len(r), q] = r
            if cur_sz[stream] and cur_sz[stream] + D * CS > GR_CAP:
                close(stream)
            cur_items[stream].append((ci, cur_sz[stream]))
            cur_toks[stream].append(blk.reshape(-1))
            cur_sz[stream] += D * CS
    close(0)
    close(1)

    tok = (np.concatenate(tok_parts) if tok_parts else np.zeros(0, np.int64))
    assert len(tok) % 128 == 0
    tok = tok.astype(np.int16)

    chunk_meta = []
    for ci, (base, A, Bn, nreal) in enumerate(chunks):
        lo = gmap.get((0, ci))
        hi = gmap.get((1, ci))
        chunk_meta.append((base, A, Bn,
                           lo[0] if lo else -1, lo[1] if lo else 0,
                           hi[0] if hi else -1, hi[1] if hi else 0))

    return dict(rowpos=rowpos, Vp=Vp, half=half, chunks=chunk_meta,
                granules=granules, tok=tok)


def _wrap16(stream):
    n = len(stream)
    assert n % 16 == 0
    w = stream.reshape(n // 16, 16).T
    return np.ascontiguousarray(np.tile(w, (8, 1))).astype(np.int16)


def _bilinear_host(plan, pos_b):
    """Per-batch: pixel-table token stream (block-colmajor order) and compact
    4-tap weights [Vp, 4] in storage-row order."""
    Vp = plan["Vp"]
    rowpos = plan["rowpos"]
    x = (pos_b[:, 0] + 1.0) * 0.5 * (W - 1)
    y = (pos_b[:, 1] + 1.0) * 0.5 * (H - 1)
    x0 = np.floor(x)
    y0 = np.floor(y)
    wx1 = (x - x0).astype(np.float32)
    wx0 = 1.0 - wx1
    wy1 = (y - y0).astype(np.float32)
    wy0 = 1.0 - wy1
    x0 = np.clip(x0.astype(np.int64), 0, W - 1)
    y0 = np.clip(y0.astype(np.int64), 0, H - 1)

    pixidx = np.zeros(Vp, np.int64)
    w4 = np.zeros((Vp, 4), np.float32)
    pixidx[rowpos] = y0 * W + x0
    w4[rowpos, 0] = wx0 * wy0
    w4[rowpos, 1] = wx1 * wy0
    w4[rowpos, 2] = wx0 * wy1
    w4[rowpos, 3] = wx1 * wy1

    stream = []
    for r0 in range(0, Vp, CVB):
        cv = min(CVB, Vp - r0)
        nv = cv // 128
        t = np.arange(cv)
        rows = r0 + (t % 128) * nv + t // 128
        stream.append(pixidx[rows])
    stream = np.concatenate(stream).astype(np.int16)
    return _wrap16(stream), w4.astype(bf16)


# ---------------------------------------------------------------- device

def _build_kernel(plan):
    import concourse.bacc as bacc
    import concourse.mybir as mybir
    from concourse.tile import TileContext

    Vp, half = plan["Vp"], plan["half"]
    chunks = plan["chunks"]
    granules = plan["granules"]
    TOK = len(plan["tok"])

    nc = bacc.Bacc("TRN2", target_bir_lowering=False, debug=False,
                   num_swdge_queues=4)
    dt = mybir.dt

    ident = nc.dram_tensor("ident", [128, 128], dt.bfloat16,
                           kind="ExternalInput")
    img4 = nc.dram_tensor("img4", [NB, NPIX, 512], dt.bfloat16,
                          kind="ExternalInput")
    vpadp = nc.dram_tensor("vpadp", [Vp, 256], dt.bfloat16,
                           kind="ExternalInput")
    bidx = nc.dram_tensor("bidx", [NB, 128, Vp // 16], dt.int16,
                          kind="ExternalInput")
    w4t = nc.dram_tensor("w4t", [NB, Vp, 4], dt.bfloat16,
                         kind="ExternalInput")
    aidx = nc.dram_tensor("aidx", [128, TOK // 16], dt.int16,
                          kind="ExternalInput")
    Pmat = nc.dram_tensor("Pmat", [4, 128, 128], dt.bfloat16,
                          kind="ExternalInput")
    outcm = nc.dram_tensor("outcm", [NB, 128, Vp], dt.float32,
                           kind="ExternalOutput")

    g_t = nc.dram_tensor("g_t", [Vp, 256], dt.bfloat16)
    a_t = [nc.dram_tensor(f"a{r}_t", [Vp, 256], dt.bfloat16)
           for r in range(2)]
    # channel-major mirrors: xc[k][x, c, row] = table_k[row, x*128+c]
    xc = [nc.dram_tensor(f"xc{k}", [2, 128, Vp], dt.bfloat16)
          for k in range(4)]

    def cm(dram_rows):
        return dram_rows.rearrange("(p u) e -> p u e", p=128)

    qn = [0]
    with TileContext(nc) as tc:
        with tc.tile_pool(name="res", bufs=1) as res:
            aidx_sb = res.tile([128, TOK // 16], dt.int16)
            nc.sync.dma_start(out=aidx_sb[:], in_=aidx[:, :])
            P_sb = res.tile([128, 4, 128], dt.bfloat16)
            nc.sync.dma_start(out=P_sb[:],
                              in_=Pmat[:, :, :].rearrange("k p m -> p k m"))
            id_sb = res.tile([128, 128], dt.bfloat16)
            nc.sync.dma_start(out=id_sb[:], in_=ident[:, :])
            zt = res.tile([128, NVB, 256], dt.bfloat16)
            nc.vector.memset(zt[:], 0.0)

            # ------------- phase B: g = bilinear + vpad -------------
            with (tc.tile_pool(name="bil", bufs=2) as bilp,
                  tc.tile_pool(name="tp", bufs=8, space="PSUM") as tpp,
                  tc.tile_pool(name="bidxp", bufs=1) as bidxp):
                bidx_sb = []
                for b in range(NB):
                    t = bidxp.tile([128, Vp // 16], dt.int16, tag=f"bi{b}")
                    nc.sync.dma_start(out=t[:], in_=bidx[b, :, :])
                    bidx_sb.append(t)
                for r0 in range(0, Vp, CVB):
                    cv = min(CVB, Vp - r0)
                    nv = cv // 128
                    gst = bilp.tile([128, nv, 2, 128], dt.bfloat16, tag="gst")
                    vp = bilp.tile([128, nv, 2, 128], dt.bfloat16, tag="vp")
                    nc.sync.dma_start(
                        out=vp[:],
                        in_=cm(vpadp[r0:r0 + cv, :])
                        .rearrange("p u (x c) -> p u x c", x=2))
                    for b in range(NB):
                        taps = bilp.tile([128, nv, 4, 128], dt.bfloat16,
                                         tag=f"taps{b}")
                        nc.gpsimd.dma_gather(
                            taps[:].rearrange("p u x c -> p u (x c)"),
                            img4[b, :, :],
                            bidx_sb[b][:, r0 // 16:(r0 + cv) // 16],
                            cv, cv, 512, single_packet=False,
                            queue_num=qn[0] % 4)
                        qn[0] += 1
                        w4sb = bilp.tile([128, nv, 4], dt.bfloat16,
                                         tag=f"w4{b}")
                        nc.sync.dma_start(out=w4sb[:],
                                          in_=cm(w4t[b, r0:r0 + cv, :]))
                        w4b = (w4sb[:].rearrange("p u x -> p (u x)")
                               .unsqueeze(2).broadcast_to((128, nv * 4, 128)))
                        t3 = taps[:].rearrange("p u x c -> p (u x) c")
                        nc.vector.tensor_mul(out=t3, in0=t3, in1=w4b)
                        tf = taps[:].rearrange("p u x c -> p u (x c)")
                        nc.vector.tensor_add(out=tf[:, :, 0:256],
                                             in0=tf[:, :, 0:256],
                                             in1=tf[:, :, 256:512])
                        nc.vector.tensor_add(out=gst[:, :, b, :],
                                             in0=tf[:, :, 0:128],
                                             in1=tf[:, :, 128:256])
                        nc.vector.tensor_add(out=gst[:, :, b, :],
                                             in0=gst[:, :, b, :],
                                             in1=vp[:, :, b, :])
                    nc.sync.dma_start(
                        out=cm(g_t.ap()[r0:r0 + cv, :]),
                        in_=gst[:].rearrange("p u x c -> p u (x c)"))
                    # channel-major mirror: PE-transpose each [128,128]
                    # (u, x) pane into PSUM, activation-copy into staging
                    # (column = storage row - r0 = p*nv + u), one big store.
                    bstage = bilp.tile([128, 2, CVB], dt.bfloat16, tag="bst")
                    for u in range(nv):
                        for x in range(2):
                            tp = tpp.tile([128, 128], dt.bfloat16, tag="tp")
                            nc.tensor.transpose(tp[:], gst[:, u, x, :],
                                                id_sb[:])
                            ocols = bstage[:].rearrange(
                                "p x (s u) -> p x s u", u=nv)[:, x, :, u]
                            nc.scalar.activation(
                                out=ocols, in_=tp[:],
                                func=mybir.ActivationFunctionType.Copy)
                    nc.sync.dma_start(
                        out=xc[0].ap()[:, :, r0:r0 + cv]
                        .rearrange("x c n -> c x n"),
                        in_=bstage[:, :, :cv])

            # ------------- phase C: a_{r+1} = A a_r -----------------
            GRB = GR_CAP // 128
            GRP = 2048          # channel-major staging columns (8 chunks)
            with (tc.tile_pool(name="gb", bufs=6) as gbp,
                  tc.tile_pool(name="tp", bufs=8, space="PSUM") as tpp,
                  tc.tile_pool(name="ac", bufs=8) as acp):
                for r in range(3):
                    src = g_t if r == 0 else a_t[r - 1]
                    dst = a_t[r] if r < 2 else None
                    dcm = xc[r + 1]
                    issued = {}
                    cstage = None
                    for (base, A, Bn, lo_g, lo_off, hi_g, hi_off) in chunks:
                        if base % GRP == 0:
                            cstage = acp.tile([128, 2, GRP], dt.bfloat16,
                                              tag="cst", bufs=3)
                        cj = base % GRP
                        for gidx, is_hi in ((lo_g, 0), (hi_g, 1)):
                            if gidx < 0 or gidx in issued:
                                continue
                            goff, gn = granules[gidx]
                            buf = gbp.tile([128, GRB, 256], dt.bfloat16,
                                           tag="gb")
                            src_ap = (src.ap()[half:, :] if is_hi
                                      else src.ap()[:, :])
                            nc.gpsimd.dma_gather(
                                buf[:, :gn // 128, :], src_ap,
                                aidx_sb[:, goff // 16:(goff + gn) // 16],
                                gn, gn, 256, single_packet=False,
                                queue_num=qn[0] % 4)
                            qn[0] += 1
                            issued[gidx] = buf
                        D = A + Bn
                        if D == 0:
                            if dst is not None:
                                nc.sync.dma_start(
                                    out=cm(dst.ap()[base:base + CS, :]),
                                    in_=zt[:])
                            for x in range(2):
                                nc.vector.tensor_copy(
                                    out=cstage[:, x, cj:cj + CS],
                                    in_=zt[:].rearrange("p u e -> p (u e)")
                                    [:, :CS])
                        else:
                            slices = []
                            if A:
                                bl = issued[lo_g]
                                o = lo_off // 128
                                slices += [bl[:, o + k * NVB:
                                              o + (k + 1) * NVB, :]
                                           for k in range(A)]
                            if Bn:
                                bh = issued[hi_g]
                                o = hi_off // 128
                                slices += [bh[:, o + k * NVB:
                                              o + (k + 1) * NVB, :]
                                           for k in range(Bn)]
                            if D == 1:
                                acc = slices[0]
                            else:
                                acct = acp.tile([128, NVB, 256], dt.bfloat16,
                                                tag="acc")
                                nc.vector.tensor_add(out=acct[:],
                                                     in0=slices[0],
                                                     in1=slices[1])
                                for s in slices[2:]:
                                    nc.vector.tensor_add(out=acct[:],
                                                         in0=acct[:], in1=s)
                                acc = acct[:]
                            if dst is not None:
                                nc.sync.dma_start(
                                    out=cm(dst.ap()[base:base + CS, :]),
                                    in_=acc)
                            av = acc.rearrange("p u (x c) -> p u x c", x=2)
                            for u in range(NVB):
                                for x in range(2):
                                    tp = tpp.tile([128, 128], dt.bfloat16,
                                                  tag="tp")
                                    nc.tensor.transpose(tp[:], av[:, u, x, :],
                                                        id_sb[:])
                                    ocols = cstage[:].rearrange(
                                        "p x (s u) -> p x s u",
                                        u=NVB)[:, x,
                                               cj // NVB:(cj + CS) // NVB, u]
                                    nc.scalar.activation(
                                        out=ocols, in_=tp[:],
                                        func=mybir.ActivationFunctionType
                                        .Copy)
                        if base % GRP == GRP - CS:
                            nc.sync.dma_start(
                                out=dcm.ap()[:, :, base + CS - GRP:
                                             base + CS]
                                .rearrange("x c n -> c x n"),
                                in_=cstage[:])

            # ------------- phase D: combine -------------------------
            with (tc.tile_pool(name="dp", bufs=3) as dp,
                  tc.tile_pool(name="ps", bufs=4, space="PSUM") as psp):
                for blk in range(0, Vp, 512):
                    for b in range(NB):
                        ps = psp.tile([128, 512], dt.float32)
                        for k in range(4):
                            xb = dp.tile([128, 512], dt.bfloat16,
                                         tag=f"x{k}b{b}")
                            eng = nc.sync if k < 2 else nc.scalar
                            eng.dma_start(
                                out=xb[:], in_=xc[k][b, :, blk:blk + 512])
                            nc.tensor.matmul(out=ps[:],
                                             lhsT=P_sb[:, k, :],
                                             rhs=xb[:],
                                             start=(k == 0), stop=(k == 3))
                        ot = dp.tile([128, 512], dt.float32, tag=f"ot{b}")
                        nc.scalar.activation(
                            out=ot[:], in_=ps[:],
                            func=mybir.ActivationFunctionType.Copy)
                        nc.sync.dma_start(out=outcm[b, :, blk:blk + 512],
                                          in_=ot[:])

    nc.compile()
    return nc


# ---------------------------------------------------------------- entry

def _make_in_maps(plan, inputs):
    Vp = plan["Vp"]
    rowpos = plan["rowpos"]
    M = [np.asarray(inputs[f"w0_{i}"], np.float64) for i in (1, 2, 3)]
    N = [np.asarray(inputs[f"w1_{i}"], np.float64) for i in (1, 2, 3)]
    P0 = M[0] + M[0] @ M[1] @ M[2]
    P1 = N[0] + N[0] @ M[1] @ M[2] + M[0] @ N[1] @ M[2] + M[0] @ M[1] @ N[2]
    P2 = N[0] @ N[1] @ M[2] + N[0] @ M[1] @ N[2] + M[0] @ N[1] @ N[2]
    P3 = N[0] @ N[1] @ N[2]
    Pm = np.ascontiguousarray(np.stack([P0, P1, P2, P3]).astype(bf16))

    img = np.asarray(inputs["img_features"], np.float32)
    pos = np.asarray(inputs["vertex_position"], np.float32)
    vpad = np.asarray(inputs["vertex_padded"], np.float32)

    # 4-pixel table per batch: row(y*W+x) = [f(y,x), f(y,x+1), f(y+1,x),
    # f(y+1,x+1)] with clamped borders (their taps always carry weight 0).
    F = img.transpose(0, 2, 3, 1)  # [B, H, W, C]
    ys, xs = np.mgrid[0:H, 0:W]
    yp = np.minimum(ys + 1, H - 1)
    xp = np.minimum(xs + 1, W - 1)
    img4_all = np.concatenate(
        [F[:, ys, xs], F[:, ys, xp], F[:, yp, xs], F[:, yp, xp]],
        axis=-1).reshape(B, NPIX, 512).astype(bf16)

    aidx_w = _wrap16(plan["tok"])

    in_maps = []
    for core in range(NCORES):
        bs = [NB * core + i for i in range(NB)]
        bidx_l, w4_l = [], []
        for b in bs:
            bi, w4 = _bilinear_host(plan, pos[b])
            bidx_l.append(bi)
            w4_l.append(w4)
        vpadp = np.zeros((Vp, 2, 128), bf16)
        for i, b in enumerate(bs):
            vpadp[rowpos, i, :] = vpad[b].astype(bf16)
        in_maps.append({
            "ident": np.eye(128, dtype=np.float32).astype(bf16),
            "img4": np.ascontiguousarray(img4_all[bs]),
            "vpadp": np.ascontiguousarray(vpadp.reshape(Vp, 256)),
            "bidx": np.ascontiguousarray(np.stack(bidx_l)),
            "w4t": np.ascontiguousarray(np.stack(w4_l)),
            "aidx": aidx_w,
            "Pmat": Pm,
        })
    return in_maps


_CACHE = {}


def kernel(**inputs):
    from concourse import bass_utils

    plan = _build_graph_plan(inputs["edges"])
    in_maps = _make_in_maps(plan, inputs)
    key = "nc"
    if key not in _CACHE:
        _CACHE[key] = _build_kernel(plan)
    nc = _CACHE[key]
    res = bass_utils.run_bass_kernel_spmd(nc, in_maps,
                                          core_ids=list(range(NCORES)))

    out = np.zeros((B, V, C), np.float32)
    for core in range(NCORES):
        oc = res.results[core]["outcm"]
        for i in range(NB):
            out[NB * core + i] = oc[i][:, plan["rowpos"]].T
    return out
